# revision 34
# baseline (speedup 1.0000x reference)
"""Trainium2 Bass kernel for the capsule-routing layer.

Math (derived from the reference):
  u_hat[b,i,j,k] = sum_d x[b,j,d] W[d, i*32+k]   (never materialized!)
  iter t: c = softmax_i(b_logits); s[i,k] = sum_j c[i,j] u_hat[i,j,k]
          o = s / sqrt(sum_k s^2 + eps); b_logits[i,j] = sum_k o[i,k] u_hat[i,j,k]
Substituting u_hat = x @ W everywhere:
  y[i,d]   = sum_j c[i,j] x[j,d]            (small matmul, K=1024)
  s[i,k]   = sum_d y[i,d] W[d, i*32+k]      (block-diagonal of y @ W)
  wtil[d,i]= sum_k W[d, i*32+k] o[i,k]      (W @ block-diag(o))
  b[i,j]   = sum_d x[j,d] wtil[d,i]         (small matmul, K=256)

Engine budget (the kernel is latency-bound, so work is spread wide):
  PE     : y-MM (4-batch tile_position quads), s-MM (cross+mask),
           wtil-MM (transposed: wtil lands directly in [d,(b,i)]), b-MM
  ACT    : Square+accum / Exp / y-evac copy (one activation table family,
           never swapped -- Sqrt would thrash it, hence quake rsqrt on DVE)
           + its HWDGE queue carries half the loads and transposes
  DVE    : mask multiply, PSUM evacuations, quake rsqrt, softmax h0
  GpSimd : softmax normalize h1 (cannot touch PSUM or do free-dim reduces)
  SP     : the other half of loads/transposes, output stores
The squash scale 1/||s|| is folded into the Exp evacuation of the
b-logits (per-partition activation scale), so it never touches the
full-width tensors. x^T (the b-MM moving operand) is fp8e4m3: it only
shapes routing logits, so the output error stays ~8e-3 vs the 2e-2 gate
while halving that tensor's share of the HBM-bound load phase.

Sharding: data-parallel, 8 batches per core; batches processed in 2 groups
of 4 stacked on SBUF partitions (partition p = 32*b + i).
"""

import numpy as np

try:
    import concourse.bass as bass
except ImportError:  # path fallback for bare environments
    import sys

    sys.path.insert(0, "/opt/trn_rl_repo")
    import concourse.bass as bass

from contextlib import ExitStack

import concourse.bacc as bacc
import concourse.tile as tile
from concourse import mybir
from concourse.bass_utils import run_bass_kernel_spmd

F32 = mybir.dt.float32
I32 = mybir.dt.int32
BF16 = mybir.dt.bfloat16
F8E4 = mybir.dt.float8e4
import os as _os

WT_FP8 = _os.environ.get("K_WT_FP8", "0") == "1"  # fallback if bf16xfp8 mixing fails
AF = mybir.ActivationFunctionType
ALU = mybir.AluOpType

NUM_CAPS = 32
DIM_CAPS = 32
D_IN = 256  # feature dim (d)
N_IN = 1024  # input capsule count (j)
IK = NUM_CAPS * DIM_CAPS  # 1024 flattened (i,k)
B_TOTAL = 64
N_CORES = 8
B_PER_CORE = 8
GB = 4  # batches per partition-group
GROUPS = B_PER_CORE // GB  # 2
EPS = 1e-7
ROUTINGS = 3


def build_program():
    nc = bacc.Bacc("TRN2", target_bir_lowering=False, debug=False)

    x_b = nc.declare_dram_parameter("x_b", [B_PER_CORE, N_IN, D_IN], BF16, isOutput=False)
    x_d = nc.declare_dram_parameter("x_d", [B_PER_CORE, D_IN, N_IN], F8E4, isOutput=False)
    w_a = nc.declare_dram_parameter("w_a", [D_IN, IK], BF16, isOutput=False)
    w_t = nc.declare_dram_parameter("w_t", [IK, D_IN], BF16, isOutput=False)
    mask_d = nc.declare_dram_parameter("mask", [128, IK], BF16, isOutput=False)
    out_d = nc.declare_dram_parameter("out", [GROUPS, 128, DIM_CAPS], F32, isOutput=True)

    with ExitStack() as ctx:
        tc = ctx.enter_context(tile.TileContext(nc))
        singles = ctx.enter_context(tc.tile_pool(name="singles", bufs=1))
        xpool = ctx.enter_context(tc.tile_pool(name="xpool", bufs=8))
        work = ctx.enter_context(tc.tile_pool(name="work", bufs=3))
        psum = ctx.enter_context(tc.tile_pool(name="ps", bufs=1, space="PSUM"))

        # ---- static tensors (ACT queue; SP queue carries group-0 x) ----
        w_a_sb = singles.tile([128, 2, IK], BF16)  # [d%128, d//128, (ik)]
        nc.scalar.dma_start(out=w_a_sb[:, :, :], in_=w_a[:, :].rearrange("(c p) n -> p c n", p=128))
        w_t_sb = singles.tile([128, 8, D_IN], BF16)  # [(ik)%128, (ik)//128, d]
        w_t_ld = lambda: nc.scalar.dma_start(
            out=w_t_sb[:, :, :], in_=w_t[:, :].rearrange("(c p) n -> p c n", p=128)
        )
        mask_sb = singles.tile([128, IK], BF16)
        mask_ld = lambda: nc.scalar.dma_start(out=mask_sb[:, :], in_=mask_d[:, :])
        cu_sb = singles.tile([128, NUM_CAPS], BF16)
        nc.vector.memset(cu_sb[:, :], 1.0 / NUM_CAPS)
        magic_sb = singles.tile([128, 1], I32)
        nc.vector.memset(magic_sb[:, :], 0x5F3759DF)
        one_i_sb = singles.tile([128, 1], I32)
        nc.vector.memset(one_i_sb[:, :], 1)

        def rsqrt_dve(nsq_ap, tagp, newton=2):
            """rn = 1/sqrt(nsq + eps): quake bit-trick + Newton, all on DVE
            (single engine avoids cross-engine semaphore hops on tiny ops)."""
            nc.vector.tensor_scalar(nsq_ap, nsq_ap, EPS, None, ALU.add)
            t_i = work.tile([128, 1], I32, tag=tagp + "i", name="nr_i")
            nc.vector.tensor_tensor(
                t_i[:, :], nsq_ap.bitcast(I32), one_i_sb[:, :], ALU.logical_shift_right
            )
            r = work.tile([128, 1], F32, tag=tagp + "r", name="nr_r")
            nc.vector.tensor_tensor(
                r[:, :].bitcast(I32), magic_sb[:, :], t_i[:, :], ALU.subtract
            )
            t2 = work.tile([128, 1], F32, tag=tagp + "t", name="nr_t")
            for _ in range(newton):
                nc.vector.tensor_mul(t2[:, :], nsq_ap, r[:, :])
                nc.vector.tensor_mul(t2[:, :], t2[:, :], r[:, :])
                nc.vector.tensor_scalar(t2[:, :], t2[:, :], -0.5, 1.5, ALU.mult, ALU.add)
                nc.vector.tensor_mul(r[:, :], r[:, :], t2[:, :])
            return r

        # ---- x loads: HBM is shared by all 8 cores, so order by first use.
        # group-0 iteration 0 needs xb0-3 + w_a immediately; everything else
        # trickles in behind them, split across the two HWDGE queues.
        xb_all, xd_all = [None] * B_PER_CORE, [None] * B_PER_CORE
        def _load_xb(bb, q):
            xb = xpool.tile([128, 8, D_IN], BF16, tag="xb", name=f"xb{bb}")
            q.dma_start(out=xb[:, :, :], in_=x_b[bb].rearrange("(c p) n -> p c n", p=128))
            xb_all[bb] = xb
        def _load_xd(bb, q):
            xd = xpool.tile([128, 2, 4, 256], F8E4, tag="xd", name=f"xd{bb}")
            q.dma_start(
                out=xd[:, :, :, :],
                in_=x_d[bb].rearrange("(c p) (s n) -> p c s n", p=128, s=4),
            )
            xd_all[bb] = xd.rearrange("p c s n -> p c (s n)")

        _load_xb(0, nc.sync)
        _load_xb(2, nc.sync)
        _load_xb(1, nc.scalar)
        _load_xb(3, nc.scalar)
        w_t_ld()
        mask_ld()
        for bb in (0, 1, 2, 3):
            _load_xd(bb, nc.sync)
        for bb in (4, 5, 6, 7):
            _load_xb(bb, nc.scalar)
        for bb in (4, 5, 6, 7):
            _load_xd(bb, nc.scalar)

        def group_stream(g):
            xb_t = [xb_all[g * GB + b] for b in range(GB)]
            xd_t = [xd_all[g * GB + b] for b in range(GB)]
            yield

            cT_sb = None  # [j%128, j//128, (4b,32i)] softmax'd coupling coeffs
            for it in range(ROUTINGS):
                last = it == ROUTINGS - 1

                # ---- y-MM: y[b,i,d] = sum_j c[b,i,j] x[b,j,d] ----
                # iteration 0 runs as two pair-quads so it can start as soon as
                # two of the four xb tiles have landed (they arrive ~2us apart
                # on the two DMA queues); later iterations use full 4-quads.
                y4_ps = psum.tile([128, D_IN], F32, tag="m32", bufs=2, name="y4_ps")
                batch_rounds = [(0, 2), (1, 3)] if it == 0 else [(0, 1, 2, 3)]
                for bs in batch_rounds:
                    for jc in range(8):
                        for b in bs:
                            lhsT = cu_sb[:, :] if it == 0 else cT_sb[:, jc, 32 * b : 32 * b + 32]
                            nc.tensor.matmul(
                                y4_ps[32 * b : 32 * b + 32, :],
                                lhsT,
                                xb_t[b][:, jc, :],
                                start=(jc == 0),
                                stop=(jc == 7),
                                tile_position=(0, 32 * b),
                                skip_group_check=True,
                            )
                yield

                # evacuate + transpose y -> [d, (4b,32i)] via the xbar
                y4_sb = work.tile([128, D_IN], BF16, tag="y4sb", name="y4_sb")
                nc.scalar.copy(y4_sb[:, :], y4_ps[:, :])
                yT_sb = work.tile([128, 2, 128], BF16, tag="yTsb", name="yT_sb")
                nc.sync.dma_start_transpose(yT_sb[:, :, :], y4_sb[:, :])
                yield

                # ---- s-MM (cross): s_cross[(b,i),(i'k)] = sum_d y[b,i,d] W[d,(i'k)] ----
                sc_ps = psum.tile([128, IK], F32, tag="big", bufs=2, name="sc_ps")
                m4_sb = work.tile([128, IK], BF16, tag="m4", name="m4_sb")
                for nh in range(2):
                    for dc in range(2):
                        nc.tensor.matmul(
                            sc_ps[:, 512 * nh : 512 * nh + 512],
                            yT_sb[:, dc, :],
                            w_a_sb[:, dc, 512 * nh : 512 * nh + 512],
                            start=(dc == 0),
                            stop=(dc == 1),
                            skip_group_check=True,
                        )
                    # mask this half (evacuates PSUM as it lands)
                    nc.vector.tensor_mul(
                        m4_sb[:, 512 * nh : 512 * nh + 512],
                        sc_ps[:, 512 * nh : 512 * nh + 512],
                        mask_sb[:, 512 * nh : 512 * nh + 512],
                    )
                yield

                if last:
                    # compact s[(b,i), k] = sum_i' masked[(b,i), (i',k)] via a
                    # contiguous fold tree (strided tensor_reduce is ~3x slower)
                    tr1 = work.tile([128, 512], F32, tag="tr1", name="tr1")
                    nc.vector.tensor_tensor(tr1[:, :], m4_sb[:, 0:512], m4_sb[:, 512:1024], ALU.add)
                    tr2 = work.tile([128, 256], F32, tag="tr2", name="tr2")
                    nc.vector.tensor_tensor(tr2[:, :], tr1[:, 0:256], tr1[:, 256:512], ALU.add)
                    tr3 = work.tile([128, 128], F32, tag="tr3", name="tr3")
                    nc.vector.tensor_tensor(tr3[:, :], tr2[:, 0:128], tr2[:, 128:256], ALU.add)
                    tr4 = work.tile([128, 64], F32, tag="tr4", name="tr4")
                    nc.vector.tensor_tensor(tr4[:, :], tr3[:, 0:64], tr3[:, 64:128], ALU.add)
                    s4c = work.tile([128, DIM_CAPS], F32, tag="s4c", name="s4c")
                    nc.vector.tensor_tensor(s4c[:, :], tr4[:, 0:32], tr4[:, 32:64], ALU.add)
                    sq_s = work.tile([128, DIM_CAPS], F32, tag="sqs", name="sq_s")
                    nsq = work.tile([128, 1], F32, tag="nsq", name="nsq")
                    nc.vector.tensor_mul(sq_s[:, :], s4c[:, :], s4c[:, :])
                    nc.vector.tensor_reduce(
                        nsq[:, :], sq_s[:, :], axis=mybir.AxisListType.X, op=ALU.add
                    )
                    rn = rsqrt_dve(nsq[:, :], "lst", newton=2)
                    o_out = work.tile([128, DIM_CAPS], F32, tag="oout", name="o_out")
                    nc.vector.tensor_scalar(o_out[:, :], s4c[:, :], rn[:, :], None, ALU.mult)
                    nc.sync.dma_start(out=out_d[g], in_=o_out[:, :])
                    return

                # ---- squash norm on GpSimd: rn4 = 1/sqrt(sum s^2 + eps) ----
                sq_scr = work.tile([128, IK], BF16, tag="scr", name="sq_scr")
                nsq4 = work.tile([128, 1], F32, tag="nsq4", name="nsq4")
                nc.scalar.activation(sq_scr[:, :], m4_sb[:, :], AF.Square, accum_out=nsq4[:, :])
                rn4 = rsqrt_dve(nsq4[:, :], "mid", newton=1)

                # ---- O^T = transpose(masked s) -> [(ik), (4b,32i)] via DMA xbar ----
                o_sb = work.tile([128, 8, 128], BF16, tag="osb", name="o_sb")
                for h, q in ((0, nc.sync), (1, nc.scalar)):
                    q.dma_start_transpose(
                        o_sb[:, 4 * h : 4 * h + 4, :], m4_sb[:, 512 * h : 512 * h + 512]
                    )
                yield

                # ---- wtil-MM (transposed): wtil[d,(b,i)] = sum_(ik) WT[(ik),d] O[(ik),(b,i)]
                # NB: both dc accumulation groups share one PSUM bank; a `start`
                # marks the whole 2KB bank pending-zero, so each dc group must
                # fully complete before the other one starts.
                wt_ps = psum.tile([128, 2, 128], F32, tag="tp2", bufs=2, name="wt_ps")
                for dc in range(2):
                    for ikc in range(8):
                        nc.tensor.matmul(
                            wt_ps[:, dc, :],
                            w_t_sb[:, ikc, 128 * dc : 128 * dc + 128],
                            o_sb[:, ikc, :],
                            start=(ikc == 0),
                            stop=(ikc == 7),
                            skip_group_check=True,
                        )
                wt_sb = work.tile([128, 2, 128], F8E4 if WT_FP8 else BF16, tag="wtsb", name="wt_sb")
                nc.vector.tensor_copy(wt_sb[:, :, :], wt_ps[:, :, :])
                yield

                # ---- b-MM: blogit[(b,i), j] = sum_d wtil[d,(b,i)] x[b][d, j] ----
                b4_ps = psum.tile([128, N_IN], F32, tag="big", bufs=2, name="b4_ps")
                e4_sb = work.tile([128, N_IN], BF16, tag="e4", name="e4_sb")
                eT_sb = work.tile([128, 8, 128], BF16, tag="eT", name="eT_sb")
                for jh in range(2):
                    if jh == 1:
                        yield
                    for dc in range(2):
                        for b in range(GB):
                            nc.tensor.matmul(
                                b4_ps[32 * b : 32 * b + 32, 512 * jh : 512 * jh + 512],
                                wt_sb[:, dc, 32 * b : 32 * b + 32],
                                xd_t[b][:, dc, 512 * jh : 512 * jh + 512],
                                start=(dc == 0),
                                stop=(dc == 1),
                                tile_position=(0, 32 * b),
                                skip_group_check=True,
                            )
                    # softmax numerator with the squash scale folded in: e = exp(rn4 * b)
                    nc.scalar.activation(
                        e4_sb[:, 512 * jh : 512 * jh + 512],
                        b4_ps[:, 512 * jh : 512 * jh + 512],
                        AF.Exp,
                        scale=rn4[:, :],
                    )
                    (nc.sync if jh == 0 else nc.scalar).dma_start_transpose(
                        eT_sb[:, 4 * jh : 4 * jh + 4, :], e4_sb[:, 512 * jh : 512 * jh + 512]
                    )
                yield

                zT_sb = work.tile([128, 8, GB], F32, tag="zT", name="zT_sb")
                rz_sb = work.tile([128, 8, GB], F32, tag="rz", name="rz_sb")
                cT_sb = work.tile([128, 8, 128], BF16, tag="cT", name="cT_sb")
                for h in range(2):
                    hc = slice(4 * h, 4 * h + 4)
                    nc.vector.tensor_reduce(
                        zT_sb[:, hc, :],
                        eT_sb[:, hc, :].rearrange("p c (b i) -> p c b i", b=GB),
                        axis=mybir.AxisListType.X,
                        op=ALU.add,
                    )
                    nc.vector.reciprocal(rz_sb[:, hc, :], zT_sb[:, hc, :])
                    eng = nc.vector if h == 0 else nc.gpsimd
                    eng.tensor_tensor(
                        cT_sb[:, hc, :].rearrange("p c (b i) -> p c b i", b=GB),
                        eT_sb[:, hc, :].rearrange("p c (b i) -> p c b i", b=GB),
                        rz_sb[:, hc, :].unsqueeze(3).broadcast_to([128, GB, GB, NUM_CAPS]),
                        ALU.mult,
                    )
                yield

        streams = [group_stream(g) for g in range(GROUPS)]
        alive = list(streams)
        while alive:
            keep = []
            for s in alive:
                try:
                    next(s)
                    keep.append(s)
                except StopIteration:
                    pass
            alive = keep

    nc.compile()
    return nc


def _host_inputs(x, W):
    import ml_dtypes

    bf16 = ml_dtypes.bfloat16
    x = np.ascontiguousarray(np.asarray(x, dtype=np.float32))
    W = np.ascontiguousarray(np.asarray(W, dtype=np.float32)).reshape(D_IN, IK)
    xT = np.ascontiguousarray(x.transpose(0, 2, 1)).astype(ml_dtypes.float8_e4m3fn)
    WT = np.ascontiguousarray(W.T).astype(bf16)
    x = x.astype(bf16)
    W = W.astype(bf16)
    q = np.arange(IK)
    p = np.arange(128)
    mask = (q[None, :] // DIM_CAPS == p[:, None] % NUM_CAPS).astype(bf16)
    return x, xT, W, WT, mask


_prog_cache = {}


def _get_program():
    if "nc" not in _prog_cache:
        _prog_cache["nc"] = build_program()
    return _prog_cache["nc"]


def kernel(x, W):
    x, xT, W, WT, mask = _host_inputs(x, W)
    nc = _get_program()
    in_maps = []
    for c in range(N_CORES):
        sl = slice(c * B_PER_CORE, (c + 1) * B_PER_CORE)
        in_maps.append(
            {
                "x_b": x[sl],
                "x_d": xT[sl],
                "w_a": W,
                "w_t": WT,
                "mask": mask,
            }
        )
    res = run_bass_kernel_spmd(nc, in_maps, core_ids=list(range(N_CORES)))
    out = np.empty((B_TOTAL, NUM_CAPS, DIM_CAPS), np.float32)
    for c in range(N_CORES):
        o = res.results[c]["out"]  # [GROUPS, 128, 32]; partition p = 32*b + i
        out[c * B_PER_CORE : (c + 1) * B_PER_CORE] = o.reshape(B_PER_CORE, NUM_CAPS, DIM_CAPS)
    return out
